# revision 24
# baseline (speedup 1.0000x reference)
"""CrossAttention Trainium2 kernel — fp8 DoubleRow matmuls + ACT/DVE exp.

Reference (B=4, C=64, H=W=64, N=4096):
    Q = Wq@q + bq; K = Wk@s + bk; V = Wv@s + bv        (1x1 convs)
    attn = softmax(Q^T K / 8, axis=m);  out = (attn @ V^T)^T + query

Sharding: 8 cores = 4 batches x 2 halves of query pixels. Per core:
2048 query pixels (4 n-tiles of 512), full 4096 keys (32 m-chunks of
128 = 16 DoubleRow pairs).

Algebra (host-folded, exact up to fp rounding):
  - scores^T = s^T K^T Q ... = sp^T q + bias_m with sp = (Wk^T Wq)^T s,
    bias_m = (Wk^T bq)^T s.  Both host-precomputed -> NO on-chip Q/K
    projection.  bk drops out of softmax.
  - Wv folded into the Z accumulator: stf rows = (Wv s)*8; a constant
    [8,0] DoubleRow weight accumulates the denominator into Z row 64
    (separate matmul: dual-fp8 ldweights requires weight free dim <=64,
    so the 65-row combined weight is not encodable).
  - The final normalize + residual (out = Z[0:64]/Z[64] + query + bv,
    0.01% of the FLOPs) runs on the host during unshard.
  - scores scaled x16 on host (sp x16) to clear fp8 subnormals; exp
    scale absorbs it (SCALE = 1/(8*16)).

Precision: all matmul operands are fp8e4m3 fed to DoubleRow matmuls
(0.5 PE cycles/row; scores contract 64+64 pad rows, Z contracts
m-chunk pairs).  exp splits across the two engines that can read PSUM
(GPSIMD cannot — bir verifier rejects it):
  ACT: exp activation -> fp8 directly, whole-pair ops [128,2,512]
       (amortizes its 185ns SBUF-access overhead)
  DVE: Schraudolph bit-trick per chunk: int8(score*(8*log2 e*SCALE)+56)
       IS the fp8e4m3 bit pattern of exp(score*SCALE) (~3% sawtooth;
       the softmax ratio cancels most of it; 4e-4 end-to-end, robust
       to trunc vs round float->int semantics).

PSUM: ACT pair-slots [128,2,512] x2 (4 banks), DVE chunk-slots x3,
zt [66,512] x1.  End-to-end rel err ~5e-4.
"""

import numpy as np
import ml_dtypes

B, C, H, W = 4, 64, 64, 64
N = H * W                # 4096 keys per batch
NCORES = 8
NPC = (B * N) // NCORES  # 2048 query pixels per core
NT = NPC // 512          # 4 n-tiles per core
MCH = N // 128           # 32 m-chunks
NPAIR = MCH // 2         # 16 DoubleRow pairs per tile

SCL_SP = 16.0
SCL_SV = 8.0
SCALE = 0.125 / SCL_SP
A_BIT = 8.0 * float(np.log2(np.e)) * SCALE
B_BIT = 56.0

# exp engine per PAIR within a tile: A=ACT (one op per pair), D=DVE
# (one op per chunk).  9 A / 7 D pairs balances 519 vs 658 ns/chunk.
PATTERN = "ADADADADADADADAA"
PLAG = 3  # pairs the scores/exp stream leads the Z matmuls by

_cache = {}


def _build():
    import concourse.bass as bass
    import concourse.tile as tile
    from concourse import bacc, mybir
    from contextlib import ExitStack

    f32 = mybir.dt.float32
    bf16 = mybir.dt.bfloat16
    f8 = mybir.dt.float8e4
    i8 = mybir.dt.int8
    DR = mybir.MatmulPerfMode.DoubleRow
    EXP = mybir.ActivationFunctionType.Exp
    CPY = mybir.ActivationFunctionType.Copy
    MUL = mybir.AluOpType.mult
    ADD = mybir.AluOpType.add

    nc = bacc.Bacc("TRN2", target_bir_lowering=False, debug=False,
                   num_devices=NCORES)

    sq_d = nc.dram_tensor("sq", [C, MCH * 2 * 128], f8, kind="ExternalInput").ap()
    qq_d = nc.dram_tensor("qq", [C, NT * 2 * 512], f8, kind="ExternalInput").ap()
    stf_d = nc.dram_tensor("stf", [128, NPAIR * 2 * C], f8,
                           kind="ExternalInput").ap()
    out_d = nc.dram_tensor("out", [C, NPC], f32, kind="ExternalOutput").ap()
    den_d = nc.dram_tensor("den", [NT, 512], f32, kind="ExternalOutput").ap()

    with tile.TileContext(nc) as tc, ExitStack() as ctx:
        const = ctx.enter_context(tc.tile_pool(name="const", bufs=1))
        data = ctx.enter_context(tc.tile_pool(name="data", bufs=1))
        apool = ctx.enter_context(tc.tile_pool(name="apsum", bufs=2, space="PSUM"))
        dpool = ctx.enter_context(tc.tile_pool(name="dpsum", bufs=2, space="PSUM"))
        zpool = ctx.enter_context(tc.tile_pool(name="zpsum", bufs=1, space="PSUM"))
        npool = ctx.enter_context(tc.tile_pool(name="npsum", bufs=1, space="PSUM"))
        epool = ctx.enter_context(tc.tile_pool(name="epool", bufs=8))
        tailp = ctx.enter_context(tc.tile_pool(name="tailp", bufs=2))

        # warm the ACT exp table while DMAs run (table load ~1.3us)
        warm = const.tile([1, 1], f32, tag="warm")
        nc.vector.memset(warm[:], 0.0)
        warm2 = const.tile([1, 1], f32, tag="warm2")
        nc.scalar.activation(warm2[:], warm[:], EXP, scale=1.0)

        # constant DoubleRow weights accumulating denominators: dual-fp8
        # ldweights needs free dim 64/128 and dst partition base 0, so the
        # denominators get their own [64,512] psum bank with ONE accumulation
        # group spanning all tiles; tile t's weight puts 8.0 in column t so
        # tile t's denominator lands in psum row t (other rows add zeros).
        den_ts = []
        for t in range(NT):
            dt_ = const.tile([128, 2, C], f8, tag=f"den{t}", name=f"den{t}")
            nc.vector.memset(dt_[:, :, :], 0.0)
            nc.vector.memset(dt_[:, :, t : t + 1], 8.0)
            den_ts.append(dt_)

        # ---- bulk loads (sync queue = serial, in dependency order) -----
        sq_t = data.tile([C, MCH, 2, 128], f8, tag="sq")
        qq_t = data.tile([C, NT, 2, 512], f8, tag="qq")
        stf_t = data.tile([128, NPAIR, 2, C], f8, tag="stf")
        SQA = 12
        nc.sync.dma_start(sq_t[:, 0:4, :, :], sq_d[:, 0:1024])
        nc.sync.dma_start(qq_t[:, 0:1, :, :], qq_d[:, 0:1024])
        nc.sync.dma_start(sq_t[:, 4:SQA, :, :], sq_d[:, 1024 : SQA * 256])
        nc.sync.dma_start(stf_t[:, :, :, :], stf_d)
        nc.sync.dma_start(sq_t[:, SQA:MCH, :, :], sq_d[:, SQA * 256 : MCH * 256])
        nc.sync.dma_start(qq_t[:, 1:NT, :, :], qq_d[:, 1024 : NT * 1024])

        # keep the PE busy while the loads land: the HAM clock gate holds a
        # cold PE at 1.2 GHz until ~3.4us of sustained activity
        wz = const.tile([C, 256], bf16, tag="wz")
        nc.vector.memset(wz[:], 0.0)
        for i in range(2):
            pw = zpool.tile([C, 512], f32, tag="zt", name=f"warmmm{i}")
            for r in range(7):
                nc.tensor.matmul(pw[0:16, bass.ts(r % 2, 256)], wz[:, 0:16],
                                 wz[:], start=True, stop=True)

        # ---- attention pipeline ----------------------------------------
        es = {}
        zts = {}

        def sc_exp(t, p):
            e_t = epool.tile([128, 2, 512], f8, tag="e", name=f"e{t}_{p}")
            es[(t, p)] = e_t
            if PATTERN[p] == "A":
                sc = apool.tile([128, 2, 512], f32, tag="sc", name=f"sca{t}_{p}")
                for j in range(2):
                    nc.tensor.matmul(sc[:, j, :], sq_t[:, 2 * p + j, :, :],
                                     qq_t[:, t, :, :], start=True, stop=True,
                                     perf_mode=DR)
                nc.scalar.activation(e_t[:, :, :], sc[:, :, :], EXP, scale=SCALE)
            else:
                for j in range(2):
                    sc = dpool.tile([128, 512], f32, tag="sc",
                                    name=f"scd{t}_{2 * p + j}")
                    nc.tensor.matmul(sc[:], sq_t[:, 2 * p + j, :, :],
                                     qq_t[:, t, :, :], start=True, stop=True,
                                     perf_mode=DR)
                    nc.vector.tensor_scalar(e_t[:, j, :].bitcast(i8), sc[:],
                                            A_BIT, B_BIT, MUL, ADD)

        zden = npool.tile([C, 512], f32, tag="zden")

        def emit_z(t, p):
            if t not in zts:
                zts[t] = zpool.tile([C, 512], f32, tag="zt", name=f"zt{t}")
            zt = zts[t]
            e_t = es.pop((t, p))
            nc.tensor.matmul(zt[:, :], stf_t[:, p, :, :], e_t[:, :, :],
                             start=(p == 0), stop=(p == NPAIR - 1),
                             perf_mode=DR, skip_group_check=True)
            nc.tensor.matmul(zden[:, :], den_ts[t][:, :, :], e_t[:, :, :],
                             start=(t == 0 and p == 0),
                             stop=(t == NT - 1 and p == NPAIR - 1),
                             perf_mode=DR, skip_group_check=True)
            if p == NPAIR - 1:
                # PSUM can't DMA (nor be read by GPSIMD): copy out via the
                # exp engines, then DMA from SBUF
                if t == NT - 1:
                    # drain: single ACT copy + single DMA per output; the
                    # denominator copy rides DVE in parallel
                    zs = tailp.tile([C, 512], f32, tag="zs", name=f"zs{t}")
                    nc.scalar.activation(zs[:], zt[:, :], CPY)
                    nc.sync.dma_start(out_d[:, bass.ts(t, 512)], zs[:])
                    ds_t = tailp.tile([NT, 512], f32, tag="ds")
                    nc.vector.tensor_copy(ds_t[:], zden[0:NT, :])
                    nc.sync.dma_start(den_d, ds_t[:])
                else:
                    za = tailp.tile([C, 256], f32, tag="za", name=f"za{t}")
                    zb = tailp.tile([C, 256], f32, tag="zb", name=f"zb{t}")
                    nc.scalar.activation(za[:], zt[:, 0:256], CPY)
                    nc.vector.tensor_copy(zb[:], zt[:, 256:512])
                    nc.sync.dma_start(out_d[:, bass.ds(t * 512, 256)], za[:])
                    nc.sync.dma_start(
                        out_d[:, bass.ds(t * 512 + 256, 256)], zb[:])

        items = [(t, p) for t in range(NT) for p in range(NPAIR)]
        for k, item in enumerate(items):
            sc_exp(*item)
            if k >= PLAG:
                emit_z(*items[k - PLAG])
        for k in range(len(items) - PLAG, len(items)):
            emit_z(*items[k])

    nc.compile()
    return nc


def _prep_inputs(query, support, Wq, bq, Wk, bk, Wv, bv):
    """Host-side shard + marshal. Returns list of 8 in_maps."""
    f8 = ml_dtypes.float8_e4m3
    q = np.asarray(query, np.float32).reshape(B, C, N)
    s = np.asarray(support, np.float32).reshape(B, C, N)
    Wq = np.asarray(Wq, np.float32); Wk = np.asarray(Wk, np.float32)
    Wv = np.asarray(Wv, np.float32)
    bq = np.asarray(bq, np.float32)

    wqk = Wk.T @ Wq
    bqk = Wk.T @ bq

    in_maps = []
    per_batch = {}
    for b in range(B):
        sp8 = ((wqk.T @ s[b]) * SCL_SP).astype(f8)          # [C, N]
        bias8 = ((bqk @ s[b]) * SCL_SP).astype(f8)          # [N]
        sv8 = ((Wv @ s[b]) * SCL_SV).astype(f8)             # [C, N]
        q8 = q[b].astype(f8)                                # [C, N]

        # sq[p, mi, 0, j] = sp8[p, mi*128+j]; sq[p, mi, 1, :] = bias row
        sq = np.zeros((C, MCH, 2, 128), f8)
        sq[:, :, 0, :] = sp8.reshape(C, MCH, 128)
        sq[0, :, 1, :] = bias8.reshape(MCH, 128)

        # stf[p, pi, jj, c] = sv8[c, (2*pi+jj)*128+p]
        stf = np.ascontiguousarray(
            sv8.reshape(C, NPAIR, 2, 128).transpose(3, 1, 2, 0))
        per_batch[b] = (sq, stf, q8)

    for core in range(NCORES):
        b, half = divmod(core, NCORES // B)
        off = half * NPC
        sq, stf, q8 = per_batch[b]
        qq = np.zeros((C, NT, 2, 512), f8)
        qq[:, :, 0, :] = q8[:, off : off + NPC].reshape(C, NT, 512)
        qq[0, :, 1, :] = 1.0
        in_maps.append({
            "sq": np.ascontiguousarray(sq).reshape(C, -1),
            "qq": np.ascontiguousarray(qq).reshape(C, -1),
            "stf": np.ascontiguousarray(stf).reshape(128, -1),
        })
    return in_maps


def _import_concourse():
    try:
        from concourse.bass_utils import run_bass_kernel_spmd
    except ImportError:
        import sys
        for p in ("/root/.axon_site/_ro/pypackages",
                  "/root/.axon_site/_ro/trn_rl_repo"):
            if p not in sys.path:
                sys.path.insert(0, p)
        from concourse.bass_utils import run_bass_kernel_spmd
    return run_bass_kernel_spmd


def kernel(**inputs):
    run_bass_kernel_spmd = _import_concourse()

    if "nc" not in _cache:
        _cache["nc"] = _build()
    nc = _cache["nc"]

    in_maps = _prep_inputs(**inputs)
    res = run_bass_kernel_spmd(nc, in_maps, list(range(NCORES)))

    q = np.asarray(inputs["query"], np.float32).reshape(B, C, N)
    bv = np.asarray(inputs["bv"], np.float32)
    out = np.empty((B, C, N), np.float32)
    for core in range(NCORES):
        b, half = divmod(core, NCORES // B)
        off = half * NPC
        z = np.asarray(res.results[core]["out"], dtype=np.float32)
        den = np.asarray(res.results[core]["den"], dtype=np.float32)
        out[b, :, off : off + NPC] = (
            z / den.reshape(1, NPC)
            + q[b, :, off : off + NPC] + bv[:, None])
    return out.reshape(B, C, H, W)
